# revision 24
# baseline (speedup 1.0000x reference)
"""Causal depthwise conv1d (B=8, C=1024, T=8192, K=4, dil=1) on 8 trn2 cores.

Sharding: batch-parallel — core j handles x[j] (1024, 8192), communication-free.

All HBM I/O rides fp16 (host rounds x, upcasts y; conv error ~6e-4 « the 2e-2
gate), halving traffic vs fp32: 32 MiB/core against the ~400 GB/s/core
achievable HBM rate (~83 us/ring for ~16.4 MiB on each HWDGE ring).
Measured: ~104-110 us HW exec (fp32 baseline: 180 us). The critical path is
the PE: 384 matmuls x ~230 ns effective (the stream rate is ~1 column/cycle
regardless of dtype, so 3 taps x 8192 cols x 8 blocks ~ 82 us is a hard
floor for this decomposition), running gap-free from ~11 us to ~96 us.

Per-core kernel (Bass/Tile), per 2048-col chunk (32 chunks):
  PE:  taps 1..3 as f16 matmuls per 512-col psum slice (lhsT = diag(w[:,k]),
       rhs = the x tile shifted k in the free dim), accumulating into
       [128, 1024] PSUM granules (4 live = all 8 banks; granule-level deps
       let each merge start after 6 matmuls instead of 12).
  ACT: one chunk-wide pass tmp = x*w0 + bias (per-partition scale/bias APs;
       N=2048 amortizes ACT's ~352-cycle fixed overhead).
  DVE: two per-granule merges ot = tmp + psum (f16 out), evicting PSUM.
  DMA: HWDGE dma_start costs ~600ns of dispatch on the issuing engine, so
       the head of the program keeps dispatch count minimal. x loads ride
       the SP ring; stores the ACT ring (per-chunk early to ramp the store
       ring, per-2-chunks in steady state, per-granule at the head/tail).
       Block 0's tap weights + a split chunk-0 load lead the SP ring so the
       PE starts ~11 us in; the bulky remaining weights ride the store
       ring's idle head window, keeping the rings at ~16.1 MiB each.
       Tile misses the "store complete before slot reuse" WAR edge for
       ACT-issued DMAs, so it is added explicitly via add_dep_helper.
"""
import numpy as np

import concourse.bacc as bacc
import concourse.mybir as mybir
from concourse.tile import TileContext
from concourse.tile import add_dep_helper
from concourse import bass_utils

B, C, T, K = 8, 1024, 8192, 4
HALO = K - 1          # causal left pad
XPAD = 13             # xt head pad: halo occupies cols 13..15 so the x DMA
                      # lands 32B-aligned (col 16) and never shares an SDMA
                      # write beat with the memset's halo bytes
P = 128               # SBUF partitions
RBLK = C // P         # 8 channel blocks per core
CHUNK = 2048          # time chunk per inner iteration
GRAN = 1024           # psum granule width (2 banks)
IOBUFS = 8            # xt pool bufs
OTBUFS = 4            # ot pool bufs (2-chunk tiles; slot-reuse WAR distance)
NCHUNK = T // CHUNK   # 4
NTOT = RBLK * NCHUNK  # 32
NPE = K - 1           # taps done on PE (1..3); tap 0 rides the ACT pass
NSMALL = 8            # chunks 1..NSMALL-1 store per-chunk (store-ring rampup)
WCOLS = RBLK * NPE * P       # packed diag-tap weights (f16)
X_DTYPE = "f16"

_cached = {}


def _build():
    nc = bacc.Bacc("TRN2", target_bir_lowering=False, debug=False)
    f32 = mybir.dt.float32
    f16 = mybir.dt.float16

    x_d = nc.dram_tensor("x", [C, T], f16, kind="ExternalInput")
    wd_d = nc.dram_tensor("wd", [P, WCOLS], f16, kind="ExternalInput")
    w0_d = nc.dram_tensor("w0", [P, RBLK], f32, kind="ExternalInput")
    b_d = nc.dram_tensor("bv", [P, RBLK], f32, kind="ExternalInput")
    y_d = nc.dram_tensor("y", [C, T], f16, kind="ExternalOutput")

    with TileContext(nc) as tc:
        with (
            tc.tile_pool(name="const", bufs=1) as cpool,
            tc.tile_pool(name="io", bufs=IOBUFS) as pool,
            tc.tile_pool(name="ox", bufs=OTBUFS) as opool,
            tc.tile_pool(name="tmp", bufs=3) as tpool,
            tc.tile_pool(name="psum", bufs=4, space="PSUM") as psum_pool,
        ):
            # packed diag weights, two dispatches (block 0 first so its
            # matmuls start ~1us earlier); tap-0 scale + bias ride two tiny
            # f32 tensors
            wt = cpool.tile([P, WCOLS], f16)
            head = NPE * P
            nc.sync.dma_start(out=wt[:, 0:head], in_=wd_d.ap()[:, 0:head])
            # bulky tap-1..3 weights for blocks 1..7 ride the store ring's
            # idle head window (stores only start ~15us in), keeping the
            # load ring 0.65 MiB lighter
            nc.scalar.dma_start(out=wt[:, head:], in_=wd_d.ap()[:, head:])
            w0t = cpool.tile([P, RBLK], f32)
            bt = cpool.tile([P, RBLK], f32)

            def wslice(r, k):
                a = (r * NPE + k - 1) * P
                return wt[:, a:a + P]

            # ot-slot store DMAs ride the ACT HWDGE ring (parallel to the SP
            # ring carrying loads). Tile misses the WAR edge "store complete
            # before slot reuse" for ACT-issued DMAs, so add it explicitly:
            # the first write into ot tile m waits on tile (m-OTBUFS)'s last
            # store (ACT-ring DMAs are FIFO, so the last store bounds them).
            tile_last_store = {}
            ot = None
            for r in range(RBLK):
                rows = slice(r * P, (r + 1) * P)
                for i in range(NCHUNK):
                    n = r * NCHUNK + i
                    xt = pool.tile([P, XPAD + HALO + CHUNK], f16, tag="xt")
                    msi = None
                    if i == 0:
                        # memset doesn't support f16; zero via uint16 view.
                        # Tile misses deps through the bitcast view, so halo
                        # readers get explicit edges below.
                        msi = nc.vector.memset(
                            xt[:, XPAD:XPAD + HALO].bitcast(mybir.dt.uint16), 0)
                        if n == 0:
                            h2 = CHUNK // 2
                            nc.sync.dma_start(
                                out=xt[:, XPAD + HALO:XPAD + HALO + h2],
                                in_=x_d.ap()[rows, 0:h2])
                            nc.sync.dma_start(
                                out=xt[:, XPAD + HALO + h2:],
                                in_=x_d.ap()[rows, h2:CHUNK])
                            nc.sync.dma_start(out=w0t, in_=w0_d.ap())
                            nc.sync.dma_start(out=bt, in_=b_d.ap())
                        else:
                            nc.sync.dma_start(out=xt[:, XPAD + HALO:],
                                              in_=x_d.ap()[rows, 0:CHUNK])
                    else:
                        nc.sync.dma_start(
                            out=xt[:, XPAD:],
                            in_=x_d.ap()[rows, i * CHUNK - HALO:(i + 1) * CHUNK])

                    pss = []
                    for g in range(2):
                        ps = psum_pool.tile([P, GRAN], f32, tag="ps")
                        pss.append(ps)
                        for sub in range(2):
                            s = g * 2 + sub
                            for k in range(1, K):
                                mm = nc.tensor.matmul(
                                    ps[:, sub * 512:(sub + 1) * 512],
                                    wslice(r, k),
                                    xt[:, XPAD + s * 512 + k:
                                        XPAD + s * 512 + k + 512],
                                    start=(k == 1), stop=(k == K - 1))
                                if msi is not None and s == 0 and k < K - 1:
                                    # taps 1..2 of the first 512 cols read
                                    # the zeroed halo via the f16 view
                                    add_dep_helper(mm.ins, msi.ins,
                                                   reason="halo memset")
                    tmp = tpool.tile([P, CHUNK], f32, tag="tmp")
                    act = nc.scalar.activation(
                        tmp, xt[:, XPAD:XPAD + CHUNK],
                        mybir.ActivationFunctionType.Identity,
                        bias=bt[:, r:r + 1], scale=w0t[:, r:r + 1])
                    if msi is not None:
                        add_dep_helper(act.ins, msi.ins, reason="halo memset")

                    m = n // 2
                    if i % 2 == 0:
                        ot = opool.tile([P, 2 * CHUNK], f16, tag="ot")
                        # every write into a reused ot slot must wait for the
                        # previous tenant's store to finish reading it
                        cur_war = tile_last_store.get(m - OTBUFS)
                    half = (i % 2) * CHUNK
                    granule_store = n == 0 or n == NTOT - 1
                    for g in range(2):
                        gsl = slice(g * GRAN, (g + 1) * GRAN)
                        osl = slice(half + g * GRAN, half + (g + 1) * GRAN)
                        tt = nc.vector.tensor_add(
                            out=ot[:, osl], in0=tmp[:, gsl], in1=pss[g])
                        if cur_war is not None:
                            add_dep_helper(
                                tt.ins, cur_war.ins,
                                reason="ot slot reuse waits for store DMA")
                        if granule_store:
                            # head/tail: store per granule so the store ring
                            # starts early / the tail drains early
                            st = nc.scalar.dma_start(
                                out=y_d.ap()[rows,
                                             i * CHUNK + g * GRAN:
                                             i * CHUNK + (g + 1) * GRAN],
                                in_=ot[:, osl])
                    if granule_store:
                        tile_last_store[m] = st
                    elif n < NSMALL or n == NTOT - 2:
                        # early chunks store per-chunk to ramp the store
                        # ring; chunk 30 stores alone since 31 is per-granule
                        st = nc.scalar.dma_start(
                            out=y_d.ap()[rows, i * CHUNK:(i + 1) * CHUNK],
                            in_=ot[:, half:half + CHUNK])
                        tile_last_store[m] = st
                    elif i % 2 == 1:
                        base = (i - 1) * CHUNK
                        st = nc.scalar.dma_start(
                            out=y_d.ap()[rows, base:base + 2 * CHUNK],
                            in_=ot)
                        tile_last_store[m] = st
    nc.compile()
    return nc


def _host_weights(w, b):
    # wd[p, (r*NPE+k-1)*P + m] = w[r*P+m, 0, k] if p == m else 0 (diag lhsT
    # blocks, taps 1..K-1); tap 0 is applied by the ACT pass via w0.
    wd = np.zeros((P, WCOLS), dtype=np.float16)
    w0 = np.ascontiguousarray(w[:, 0, 0].reshape(RBLK, P).T).astype(np.float32)
    bv = np.ascontiguousarray(b.reshape(RBLK, P).T).astype(np.float32)
    m = np.arange(P)
    for r in range(RBLK):
        for k in range(1, K):
            wd[m, (r * NPE + k - 1) * P + m] = \
                w[r * P + m, 0, k].astype(np.float16)
    return wd, w0, bv


def kernel(x, w, b):
    x = np.asarray(x, dtype=np.float32)
    w = np.asarray(w, dtype=np.float32)
    b = np.asarray(b, dtype=np.float32)

    if "nc" not in _cached:
        _cached["nc"] = _build()
    nc = _cached["nc"]

    wd, w0, bv = _host_weights(w, b)
    x16 = x.astype(np.float16)
    in_maps = [
        {"x": np.ascontiguousarray(x16[j]), "wd": wd, "w0": w0, "bv": bv}
        for j in range(B)
    ]
    res = bass_utils.run_bass_kernel_spmd(nc, in_maps, core_ids=list(range(B)))
    return np.stack([r["y"] for r in res.results], axis=0).astype(np.float32)
